# revision 5
# baseline (speedup 1.0000x reference)
"""DglGraphConvolution Trainium2 kernel — dense-adjacency matmul aggregation.

out = (A @ text) @ W / (deg+1) + bias, per graph (N=4096 nodes, F=128).

Per graph:
  1. text [N, F] f32 DMA'd in 4 pieces (ACT HWDGE ring), cast bf16 on DVE
     -> textb [128, ws, 128] (partition = node % 128 within window ws).
  2. Aggregation vs the dense adjacency-count matrix AT[src, dst]
     (bincount of edges, exact small ints, shipped fp8_e4m3, 2 MB per
     512-dst chunk on the SP HWDGE ring): 32 accumulating matmuls
       aggT[fin, dst] += textb[:, ws, :].T @ AT[ws, chunk]  (bf16 x fp8)
     into one PSUM bank; evacuated to SBUF as bf16 (ACT).
  3. W-apply: per 128-dst window, out_ps[dst, f] = aggTb[:, w].T @ Wb
     -- output lands in natural [node, f] orientation. Emitted one chunk
     behind the aggregation so the PE never waits on the evacuation.
  4. Epilogue: ACT scales by rec[:, w] (per-partition scalar = 1/(deg+1)),
     DVE adds the replicated bias and writes bf16 into o_full.
  5. o_full DMA'd out per half graph (bf16); host upcasts + un-shuffles.

Host-side work is sharding plus re-encoding of the edge index lists into
adjacency counts / degree counts (np.bincount) and layout shuffles — no
arithmetic touches model float data.
"""

import numpy as np

B, N, E, F = 16, 4096, 131072, 128
NCORES = 8
GPC = B // NCORES  # graphs per core
W = 128  # node window (matmul contraction tile)
NW = N // W  # 32
DC = 512  # dst columns per chunk (one PSUM bank of f32)
NCHUNK = N // DC  # 8
WPC = DC // W  # windows per chunk = 4
TP = 4  # text DMA pieces
PREFETCH_AT = 3  # chunk index of graph g at which graph g+1 text is fetched

_cache = {}


def _build_program():
    from contextlib import ExitStack

    import concourse.bacc as bacc
    import concourse.tile as tile
    from concourse import mybir
    from concourse._compat import get_trn_type

    f32 = mybir.dt.float32
    bf16 = mybir.dt.bfloat16
    fp8 = mybir.dt.float8e4

    nc = bacc.Bacc(get_trn_type() or "TRN2", target_bir_lowering=False, debug=False)

    # text in window layout: [g, p, ws, f] = text[g, ws*128+p, f]
    text_d = nc.dram_tensor("textw", [GPC, W, NW, F], f32, kind="ExternalInput")
    w_d = nc.dram_tensor("weight", [F, F], f32, kind="ExternalInput")
    bias_d = nc.dram_tensor("biasrep", [W, F], f32, kind="ExternalInput")
    # adjacency counts AT[src, dst] as [chunk, src_row, ws, dst_col]
    at_d = nc.dram_tensor("at8", [GPC, NCHUNK, W, NW, DC], fp8, kind="ExternalInput")
    # degree counts in window layout [p, w] = deg[w*128+p]
    deg_d = nc.dram_tensor("degw", [GPC, W, NW], f32, kind="ExternalInput")
    # out in window layout: [g, p, w, f] = out[g, w*128+p, f], bf16
    out_d = nc.dram_tensor("out", [GPC, W, NW, F], bf16, kind="ExternalOutput")

    with tile.TileContext(nc) as tc, ExitStack() as ctx:
        const = ctx.enter_context(tc.tile_pool(name="const", bufs=1))
        tfpool = ctx.enter_context(tc.tile_pool(name="tf", bufs=3))
        tbpool = ctx.enter_context(tc.tile_pool(name="tb", bufs=2))
        atpool = ctx.enter_context(tc.tile_pool(name="atp", bufs=5))
        gpool = ctx.enter_context(tc.tile_pool(name="gp", bufs=3))
        rpool = ctx.enter_context(tc.tile_pool(name="rp", bufs=2))
        opool = ctx.enter_context(tc.tile_pool(name="op", bufs=2))
        apsum = ctx.enter_context(tc.tile_pool(name="apsum", bufs=2, space="PSUM"))
        opsum = ctx.enter_context(tc.tile_pool(name="opsum", bufs=4, space="PSUM"))

        w_sb = const.tile([F, F], f32)
        nc.scalar.dma_start(w_sb[:], w_d[:, :])
        w_bf = const.tile([F, F], bf16)
        nc.vector.tensor_copy(w_bf[:], w_sb[:])
        bias_sb = const.tile([W, F], f32)
        nc.scalar.dma_start(bias_sb[:], bias_d[:, :])

        state = {}  # per-graph tiles, filled by prefetch

        def prefetch(g):
            # deg -> rec, and text pieces (ACT ring), cast on DVE
            deg_sb = rpool.tile([W, NW], f32, tag="deg")
            nc.scalar.dma_start(deg_sb[:], deg_d[g])
            rec = rpool.tile([W, NW], f32, tag="rec")
            nc.vector.tensor_scalar_add(rec[:], deg_sb[:], 1.0)
            nc.vector.reciprocal(rec[:], rec[:])
            textb = tbpool.tile([W, NW, F], bf16, tag="tb")
            wpp = NW // TP
            for p in range(TP):
                textf = tfpool.tile([W, wpp, F], f32, tag="tf")
                nc.scalar.dma_start(textf[:], text_d[g, :, wpp * p : wpp * (p + 1), :])
                nc.vector.tensor_copy(textb[:, wpp * p : wpp * (p + 1), :], textf[:])
            state[g] = (rec, textb)

        prefetch(0)
        for g in range(GPC):
            rec, textb = state.pop(g)
            o_full = opool.tile([W, NW, F], bf16, tag="of")
            pend = []  # (aggb_tile, chunk_idx)

            def wapply(entry, g=g, rec=rec, o_full=o_full):
                aggb, c = entry
                for q in range(WPC):
                    w = c * WPC + q
                    out_ps = opsum.tile([W, F], f32, tag="ops")
                    nc.tensor.matmul(
                        out=out_ps[:],
                        lhsT=aggb[:, W * q : W * (q + 1)],
                        rhs=w_bf[:],
                        start=True,
                        stop=True,
                    )
                    o_tmp = gpool.tile([W, F], f32, tag="otmp")
                    nc.scalar.activation(
                        o_tmp[:],
                        out_ps[:],
                        mybir.ActivationFunctionType.Identity,
                        bias=0.0,
                        scale=rec[:, w : w + 1],
                    )
                    nc.vector.tensor_add(o_full[:, w, :], o_tmp[:], bias_sb[:])
                if c % (NCHUNK // 2) == NCHUNK // 2 - 1:
                    half = c // (NCHUNK // 2)
                    lo, hi = half * NW // 2, (half + 1) * NW // 2
                    nc.scalar.dma_start(out_d[g, :, lo:hi, :], o_full[:, lo:hi, :])

            for c in range(NCHUNK):
                at_sb = atpool.tile([W, NW, DC], fp8, tag="at")
                nc.sync.dma_start(at_sb[:], at_d[g, c])
                agg_ps = apsum.tile([F, DC], f32, tag="agg")
                for ws in range(NW):
                    nc.tensor.matmul(
                        out=agg_ps[:],
                        lhsT=textb[:, ws, :],
                        rhs=at_sb[:, ws, :],
                        start=(ws == 0),
                        stop=(ws == NW - 1),
                    )
                aggb = gpool.tile([F, DC], bf16, tag="aggb")
                nc.scalar.activation(
                    aggb[:], agg_ps[:], mybir.ActivationFunctionType.Copy
                )
                pend.append((aggb, c))
                if c == PREFETCH_AT and g + 1 < GPC:
                    prefetch(g + 1)
                if len(pend) > 1:
                    wapply(pend.pop(0))
            while pend:
                wapply(pend.pop(0))

    nc.compile()
    return nc


def kernel(text, weight, bias, edge_src, edge_dst):
    import ml_dtypes

    text = np.asarray(text, dtype=np.float32)
    weight = np.asarray(weight, dtype=np.float32)
    bias = np.asarray(bias, dtype=np.float32)
    edge_src = np.asarray(edge_src, dtype=np.int64)
    edge_dst = np.asarray(edge_dst, dtype=np.int64)

    if "nc" not in _cache:
        _cache["nc"] = _build_program()
    nc = _cache["nc"]

    bias_rep = np.tile(bias[None, :], (W, 1)).astype(np.float32)

    in_maps = []
    for k in range(NCORES):
        at8 = np.empty((GPC, NCHUNK, W, NW, DC), dtype=ml_dtypes.float8_e4m3)
        degw = np.empty((GPC, W, NW), dtype=np.float32)
        textw = np.empty((GPC, W, NW, F), dtype=np.float32)
        for g in range(GPC):
            b = k * GPC + g
            src, dst = edge_src[b], edge_dst[b]
            cnt = np.bincount(src * N + dst, minlength=N * N)
            assert cnt.max() <= 15, f"edge multiplicity overflow: {cnt.max()}"
            # AT[src, dst] -> [chunk, src_row, ws, dst_col]
            at = cnt.astype(np.float32).reshape(NW, W, NCHUNK, DC)
            at8[g] = at.transpose(2, 1, 0, 3).astype(ml_dtypes.float8_e4m3)
            degw[g] = (
                np.bincount(dst, minlength=N).astype(np.float32).reshape(NW, W).T
            )
            textw[g] = text[b].reshape(NW, W, F).transpose(1, 0, 2)
        in_maps.append(
            {
                "textw": textw,
                "weight": weight,
                "biasrep": bias_rep,
                "at8": at8,
                "degw": degw,
            }
        )

    _cache["in_maps"] = in_maps

    from concourse.bass_utils import run_bass_kernel_spmd

    res = run_bass_kernel_spmd(nc, in_maps, list(range(NCORES)))
    # res out: [GPC, 128, NW, F] bf16 window layout -> [GPC, N, F] f32
    out = np.concatenate(
        [
            res.results[k]["out"]
            .astype(np.float32)
            .transpose(0, 2, 1, 3)
            .reshape(GPC, N, F)
            for k in range(NCORES)
        ],
        axis=0,
    )
    return np.ascontiguousarray(out)
